# revision 3
# baseline (speedup 1.0000x reference)
"""Trainium2 Bass kernel for nn_DiscreteExactLoss (joint-entropy loss).

Reference computation:
    soft_assign[b, r, :] = [1 - a[b,r], a[b,r]]          (K=2, R=10)
    joint_p[b, s]  = prod_r soft_assign[b, r, s_r]       (s in [0, 1024))
    p_a            = mean_b joint_p                       [1024]
    out            = sum_s p_a * log2(p_a)               (scalar, ~-10)

Device algorithm (per core, data-parallel over B across 8 cores):
    Accumulate MULTILINEAR MOMENTS m_T = sum_b prod_{r in T} a[b, r] for
    all 1024 subsets T. Moments factor over a 5+5 variable split:
    m_{T1 u T2} = sum_b MA[b,T1]*MC[b,T2], where MA/MC are the 32
    subset-products of each 5-var half. The 32x32 outer product summed
    over b is a TensorEngine matmul accumulated in PSUM.

    Engine split (v2):
      - ScalarE: f32->bf16 cast of the 10 per-sample activities straight
        into the singleton slots (m = 1,2,4,8,16) of the mask table; the
        DVE never touches raw f32 data.
      - DVE: the remaining 26-per-half subset products via 4 broadcast-
        multiply levels (bf16 tensor_tensor at 2x mode), split into two
        c-halves so the PE can start early.
      - PE: per 128-sample chunk, 4x column-tiled matmuls (tile_position
        (0, 32j)) run concurrently on the 4 col groups of the array; each
        col group accumulates its own [32,32] partial in one PSUM bank.
      - Host: sum 8 cores x 4 col-group partials, Mobius transform
        (moments -> probabilities), p*log2(p) reduction (~30k flops).
"""

import math
import sys

import numpy as np

if "/opt/trn_rl_repo" not in sys.path:
    sys.path.insert(0, "/opt/trn_rl_repo")

B_FULL = 131072
R_FULL = 10
N_CORES = 8
B_LOC = B_FULL // N_CORES  # 16384
P = 128                    # SBUF partitions; samples per matmul chunk
C = B_LOC // P             # 128 sample-chunks per core (c dim)
NCH = 2                    # c-halves for pipelining
CH = C // NCH              # 64 c per half
QUAD = 4                   # chunks per col-tiled matmul quartet

_NC_CACHE = {}


def _build_module():
    if "nc" in _NC_CACHE:
        return _NC_CACHE["nc"]

    from concourse import bacc, bass, mybir, tile

    f32 = mybir.dt.float32
    bf16 = mybir.dt.bfloat16

    nc = bacc.Bacc("TRN2", target_bir_lowering=False, debug=False)

    act = nc.dram_tensor("act", [B_LOC, R_FULL], f32, kind="ExternalInput")
    msum = nc.dram_tensor("msum", [P, 32], f32, kind="ExternalOutput")

    # dram view [p, c, r]: sample b = p*C + c
    act_pcr = act.ap().rearrange("(p c) r -> p c r", p=P)
    # dram view splitting r into halves h (vars 5h+l): [p, c, h, l]
    act_phl = act.ap().rearrange("(p c) (h l) -> p c h l", p=P, h=2)

    with tile.TileContext(nc) as tc:
        with (
            tc.tile_pool(name="a0", bufs=1) as a0_pool,
            tc.tile_pool(name="mac", bufs=1) as mac_pool,
            tc.tile_pool(name="outp", bufs=1) as out_pool,
            tc.tile_pool(name="psum", bufs=1, space=bass.MemorySpace.PSUM) as psum_pool,
        ):
            # raw activity, f32, natural layout [p, c, r]
            a0 = a0_pool.tile([P, C, R_FULL], f32)
            # mask table: mac[p, h, m, c] = prod of half-h vars in mask m
            mac = mac_pool.tile([P, 2, 32, C], bf16)
            psum_acc = psum_pool.tile([P, 32], f32)

            # m=0 column := 1.0 (empty product); used only by the matmul.
            # GpSimd is otherwise idle; keep the DVE free for the cascade.
            nc.gpsimd.memset(mac[:, :, 0:1, :], 1.0)

            for ch in range(NCH):
                cs = slice(ch * CH, (ch + 1) * CH)

                # ---- load raw fp32 activity for this c-half ----
                nc.sync.dma_start(out=a0[:, cs, :], in_=act_pcr[:, cs, :])

                # ---- GpSimd: cast singletons straight into mac ----
                # (ScalarE would pay a ~2.7us ACT table load; GpSimd is
                # idle and casts f32->bf16 at ~1 elem/cycle.)
                # m in {1,2} <- vars l in {0,1}; m in {4,8} <- l in {2,3}
                # (m-step 4 matches l-step 1); m=16 <- l=4.
                a_half = a0[:, cs, :].rearrange("p c (h l) -> p h l c", h=2)
                nc.gpsimd.tensor_copy(mac[:, :, 1:3, cs], a_half[:, :, 0:2, :])
                nc.gpsimd.tensor_copy(mac[:, :, 4:12:4, cs], a_half[:, :, 2:4, :])
                nc.gpsimd.tensor_copy(mac[:, :, 16:17, cs], a_half[:, :, 4:5, :])

                # ---- DVE: broadcast-multiply cascade ----
                # level l: m in (2^l, 2^(l+1)) := m' in [1, 2^l) * a_{var l}
                for lvl in range(1, 4):
                    j = 1 << lvl
                    a_bc = mac[:, :, j:j + 1, cs].broadcast_to([P, 2, j - 1, CH])
                    nc.vector.tensor_tensor(
                        mac[:, :, j + 1:2 * j, cs],
                        mac[:, :, 1:j, cs],
                        a_bc,
                        mybir.AluOpType.mult,
                    )
                # level 4 split into c-quarters so the PE can start early
                for sub in range(2):
                    qs = slice(ch * CH + sub * (CH // 2),
                               ch * CH + (sub + 1) * (CH // 2))
                    a_bc = mac[:, :, 16:17, qs].broadcast_to([P, 2, 15, CH // 2])
                    nc.vector.tensor_tensor(
                        mac[:, :, 17:32, qs],
                        mac[:, :, 1:16, qs],
                        a_bc,
                        mybir.AluOpType.mult,
                    )

                    # ---- PE: col-tiled moment accumulation ----
                    # quartet q covers chunks cc..cc+3; col group jj handles
                    # chunk cc+jj, accumulating into psum partitions 32jj..
                    for q in range(CH // 2 // QUAD):
                        cc = qs.start + q * QUAD
                        gq = cc // QUAD
                        for jj in range(QUAD):
                            nc.tensor.matmul(
                                psum_acc[32 * jj:32 * jj + 32, :],
                                mac[:, 0, :, cc + jj],   # lhsT [K=128, M=32]
                                mac[:, 1, :, cc + jj],   # rhs  [K=128, N=32]
                                start=(gq == 0),
                                stop=(gq == C // QUAD - 1),
                                tile_position=(0, 32 * jj),
                            )

            out_sb = out_pool.tile([P, 32], f32)
            nc.vector.tensor_copy(out_sb[:, :], psum_acc[:, :])
            nc.sync.dma_start(out=msum[:, :], in_=out_sb[:, :])

    # Bacc modules carry virtual registers until compile() runs; the
    # bass2jax/PJRT path serializes nc as-is, so allocate them now.
    nc.compile()
    _NC_CACHE["nc"] = nc
    return nc


def _ensure_ntff_hook():
    """The agent image's antenv package lacks axon_hooks; synthesize it so
    run_bass_kernel_spmd(trace=True) can find the NTFF profile hook."""
    import types

    try:
        from antenv.axon_hooks import get_axon_ntff_profile_hook  # noqa: F401
        return
    except ImportError:
        pass
    import antenv

    mod = types.ModuleType("antenv.axon_hooks")
    state = {"hook": None}
    mod.set_axon_ntff_profile_hook = lambda h: state.__setitem__("hook", h)
    mod.get_axon_ntff_profile_hook = lambda: state["hook"]
    antenv.axon_hooks = mod
    sys.modules["antenv.axon_hooks"] = mod

    try:
        from trn_agent_boot.trn_boot import _ntff_profile_via_ctypes

        hook = _ntff_profile_via_ctypes("/opt/axon/libaxon_pjrt.so")
        if hook is not None:
            mod.set_axon_ntff_profile_hook(hook)
    except Exception:
        pass


def _run_on_device(activity, trace=False):
    from concourse.bass_utils import run_bass_kernel_spmd

    if trace:
        _ensure_ntff_hook()
    nc = _build_module()
    shards = np.ascontiguousarray(activity.astype(np.float32)).reshape(
        N_CORES, B_LOC, R_FULL
    )
    in_maps = [{"act": np.ascontiguousarray(shards[i])} for i in range(N_CORES)]
    res = run_bass_kernel_spmd(
        nc, in_maps, core_ids=list(range(N_CORES)), trace=trace
    )
    return res


def _finish_on_host(per_core_msums):
    # total moment sums over all B samples; fold the 4 col-group partials
    msum = np.zeros((32, 32), dtype=np.float64)
    for part in per_core_msums:
        p128 = part.astype(np.float64).reshape(4, 32, 32)
        msum += p128.sum(axis=0)
    m = (msum / B_FULL).reshape(-1)  # [1024] mean moments

    # Mobius transform per bit: p(bit=0) = m(without) - m(with)
    p = m.copy()
    idx = np.arange(1024)
    for bit in range(10):
        step = 1 << bit
        lo = idx[(idx & step) == 0]
        p[lo] = p[lo] - p[lo | step]

    p = p.astype(np.float32)
    p_safe = np.clip(p, 1e-12, None)
    log_k_p = np.log(p_safe) / math.log(2.0)
    joint_h = -np.sum(p * log_k_p)
    return np.array(-joint_h, dtype=np.float32)


def kernel(activity):
    res = _run_on_device(activity, trace=False)
    return _finish_on_host([r["msum"] for r in res.results])


def kernel_profiled(activity):
    """Like kernel() but with NTFF tracing; returns (output, exec_time_ns)."""
    res = _run_on_device(activity, trace=True)
    out = _finish_on_host([r["msum"] for r in res.results])
    return out, res.exec_time_ns


# revision 4
# speedup vs baseline: 1.2022x; 1.2022x over previous
"""Trainium2 Bass kernel for nn_DiscreteExactLoss (joint-entropy loss).

Reference computation:
    soft_assign[b, r, :] = [1 - a[b,r], a[b,r]]          (K=2, R=10)
    joint_p[b, s]  = prod_r soft_assign[b, r, s_r]       (s in [0, 1024))
    p_a            = mean_b joint_p                       [1024]
    out            = sum_s p_a * log2(p_a)               (scalar, ~-10)

Device algorithm (per core, data-parallel over B across 8 cores):
    Accumulate MULTILINEAR MOMENTS m_T = sum_b prod_{r in T} a[b, r] for
    all 1024 subsets T. Moments factor over a 5+5 variable split:
    m_{T1 u T2} = sum_b MA[b,T1]*MC[b,T2], where MA/MC are the 32
    subset-products of each 5-var half. The 32x32 outer product summed
    over b is a TensorEngine matmul accumulated in PSUM.

    Engine split (v2):
      - ScalarE: f32->bf16 cast of the 10 per-sample activities straight
        into the singleton slots (m = 1,2,4,8,16) of the mask table; the
        DVE never touches raw f32 data.
      - DVE: the remaining 26-per-half subset products via 4 broadcast-
        multiply levels (bf16 tensor_tensor at 2x mode), split into two
        c-halves so the PE can start early.
      - PE: per 128-sample chunk, 4x column-tiled matmuls (tile_position
        (0, 32j)) run concurrently on the 4 col groups of the array; each
        col group accumulates its own [32,32] partial in one PSUM bank.
      - Host: sum 8 cores x 4 col-group partials, Mobius transform
        (moments -> probabilities), p*log2(p) reduction (~30k flops).
"""

import math
import sys

import numpy as np

if "/opt/trn_rl_repo" not in sys.path:
    sys.path.insert(0, "/opt/trn_rl_repo")

B_FULL = 131072
R_FULL = 10
N_CORES = 8
B_LOC = B_FULL // N_CORES  # 16384
P = 128                    # SBUF partitions; samples per matmul chunk
C = B_LOC // P             # 128 sample-chunks per core (c dim)
NCH = 2                    # c-halves for pipelining
CH = C // NCH              # 64 c per half
QUAD = 4                   # chunks per col-tiled matmul quartet

_NC_CACHE = {}


def _build_module():
    if "nc" in _NC_CACHE:
        return _NC_CACHE["nc"]

    from concourse import bacc, bass, mybir, tile

    f32 = mybir.dt.float32
    bf16 = mybir.dt.bfloat16

    nc = bacc.Bacc("TRN2", target_bir_lowering=False, debug=False)

    act = nc.dram_tensor("act", [B_LOC, R_FULL], f32, kind="ExternalInput")
    msum = nc.dram_tensor("msum", [P, 32], f32, kind="ExternalOutput")

    # dram view [p, c, r]: sample b = p*C + c
    act_pcr = act.ap().rearrange("(p c) r -> p c r", p=P)
    # dram view splitting r into halves h (vars 5h+l): [p, c, h, l]
    act_phl = act.ap().rearrange("(p c) (h l) -> p c h l", p=P, h=2)

    with tile.TileContext(nc) as tc:
        with (
            tc.tile_pool(name="a0", bufs=1) as a0_pool,
            tc.tile_pool(name="mac", bufs=1) as mac_pool,
            tc.tile_pool(name="outp", bufs=1) as out_pool,
            tc.tile_pool(name="psum", bufs=1, space=bass.MemorySpace.PSUM) as psum_pool,
        ):
            # raw activity, f32, natural layout [p, c, r]
            a0 = a0_pool.tile([P, C, R_FULL], f32)
            # mask table: mac[p, h, m, c] = prod of half-h vars in mask m
            mac = mac_pool.tile([P, 2, 32, C], bf16)
            warm = mac_pool.tile([P, 1], bf16)
            psum_acc = psum_pool.tile([P, 32], f32)

            # Warm the ScalarE activation tables immediately: the first
            # ACTIVATE pays ~2.7us of ACT_TABLE_LOAD + drain, which this
            # dummy op absorbs while the DMAs are still in flight.
            nc.scalar.copy(warm[:, :], warm[:, :])

            # m=0 column := 1.0 (empty product); used only by the matmul.
            # GpSimd is otherwise idle; keep the DVE free for the cascade.
            nc.gpsimd.memset(mac[:, :, 0:1, :], 1.0)

            for ch in range(NCH):
                cs = slice(ch * CH, (ch + 1) * CH)

                # ---- load raw fp32 activity for this c-half ----
                nc.sync.dma_start(out=a0[:, cs, :], in_=act_pcr[:, cs, :])

                # ---- cast singletons straight into mac ----
                # m in {1,2} <- vars l in {0,1}; m in {4,8} <- l in {2,3}
                # (m-step 4 matches l-step 1); m=16 <- l=4.
                # First half: DVE casts the level-gating singletons itself
                # (ScalarE is still cold from the table load); ScalarE
                # handles {16} plus the whole second half, hidden under
                # the first half's cascade.
                a_half = a0[:, cs, :].rearrange("p c (h l) -> p h l c", h=2)
                if ch == 0:
                    nc.vector.tensor_copy(mac[:, :, 1:3, cs], a_half[:, :, 0:2, :])
                    nc.vector.tensor_copy(mac[:, :, 4:12:4, cs], a_half[:, :, 2:4, :])
                    nc.scalar.copy(mac[:, :, 16:17, cs], a_half[:, :, 4:5, :])
                else:
                    nc.scalar.copy(mac[:, :, 1:3, cs], a_half[:, :, 0:2, :])
                    nc.scalar.copy(mac[:, :, 4:12:4, cs], a_half[:, :, 2:4, :])
                    nc.scalar.copy(mac[:, :, 16:17, cs], a_half[:, :, 4:5, :])

                # ---- DVE: broadcast-multiply cascade ----
                # level l: m in (2^l, 2^(l+1)) := m' in [1, 2^l) * a_{var l}
                for lvl in range(1, 4):
                    j = 1 << lvl
                    a_bc = mac[:, :, j:j + 1, cs].broadcast_to([P, 2, j - 1, CH])
                    nc.vector.tensor_tensor(
                        mac[:, :, j + 1:2 * j, cs],
                        mac[:, :, 1:j, cs],
                        a_bc,
                        mybir.AluOpType.mult,
                    )
                # level 4 split into c-quarters so the PE can start early
                for sub in range(2):
                    qs = slice(ch * CH + sub * (CH // 2),
                               ch * CH + (sub + 1) * (CH // 2))
                    a_bc = mac[:, :, 16:17, qs].broadcast_to([P, 2, 15, CH // 2])
                    nc.vector.tensor_tensor(
                        mac[:, :, 17:32, qs],
                        mac[:, :, 1:16, qs],
                        a_bc,
                        mybir.AluOpType.mult,
                    )

                    # ---- PE: col-tiled moment accumulation ----
                    # quartet q covers chunks cc..cc+3; col group jj handles
                    # chunk cc+jj, accumulating into psum partitions 32jj..
                    for q in range(CH // 2 // QUAD):
                        cc = qs.start + q * QUAD
                        gq = cc // QUAD
                        for jj in range(QUAD):
                            nc.tensor.matmul(
                                psum_acc[32 * jj:32 * jj + 32, :],
                                mac[:, 0, :, cc + jj],   # lhsT [K=128, M=32]
                                mac[:, 1, :, cc + jj],   # rhs  [K=128, N=32]
                                start=(gq == 0),
                                stop=(gq == C // QUAD - 1),
                                tile_position=(0, 32 * jj),
                            )

            out_sb = out_pool.tile([P, 32], f32)
            nc.vector.tensor_copy(out_sb[:, :], psum_acc[:, :])
            nc.sync.dma_start(out=msum[:, :], in_=out_sb[:, :])

    # Bacc modules carry virtual registers until compile() runs; the
    # bass2jax/PJRT path serializes nc as-is, so allocate them now.
    nc.compile()
    _NC_CACHE["nc"] = nc
    return nc


def _ensure_ntff_hook():
    """The agent image's antenv package lacks axon_hooks; synthesize it so
    run_bass_kernel_spmd(trace=True) can find the NTFF profile hook."""
    import types

    try:
        from antenv.axon_hooks import get_axon_ntff_profile_hook  # noqa: F401
        return
    except ImportError:
        pass
    import antenv

    mod = types.ModuleType("antenv.axon_hooks")
    state = {"hook": None}
    mod.set_axon_ntff_profile_hook = lambda h: state.__setitem__("hook", h)
    mod.get_axon_ntff_profile_hook = lambda: state["hook"]
    antenv.axon_hooks = mod
    sys.modules["antenv.axon_hooks"] = mod

    try:
        from trn_agent_boot.trn_boot import _ntff_profile_via_ctypes

        hook = _ntff_profile_via_ctypes("/opt/axon/libaxon_pjrt.so")
        if hook is not None:
            mod.set_axon_ntff_profile_hook(hook)
    except Exception:
        pass


def _run_on_device(activity, trace=False):
    from concourse.bass_utils import run_bass_kernel_spmd

    if trace:
        _ensure_ntff_hook()
    nc = _build_module()
    shards = np.ascontiguousarray(activity.astype(np.float32)).reshape(
        N_CORES, B_LOC, R_FULL
    )
    in_maps = [{"act": np.ascontiguousarray(shards[i])} for i in range(N_CORES)]
    res = run_bass_kernel_spmd(
        nc, in_maps, core_ids=list(range(N_CORES)), trace=trace
    )
    return res


def _finish_on_host(per_core_msums):
    # total moment sums over all B samples; fold the 4 col-group partials
    msum = np.zeros((32, 32), dtype=np.float64)
    for part in per_core_msums:
        p128 = part.astype(np.float64).reshape(4, 32, 32)
        msum += p128.sum(axis=0)
    m = (msum / B_FULL).reshape(-1)  # [1024] mean moments

    # Mobius transform per bit: p(bit=0) = m(without) - m(with)
    p = m.copy()
    idx = np.arange(1024)
    for bit in range(10):
        step = 1 << bit
        lo = idx[(idx & step) == 0]
        p[lo] = p[lo] - p[lo | step]

    p = p.astype(np.float32)
    p_safe = np.clip(p, 1e-12, None)
    log_k_p = np.log(p_safe) / math.log(2.0)
    joint_h = -np.sum(p * log_k_p)
    return np.array(-joint_h, dtype=np.float32)


def kernel(activity):
    res = _run_on_device(activity, trace=False)
    return _finish_on_host([r["msum"] for r in res.results])


def kernel_profiled(activity):
    """Like kernel() but with NTFF tracing; returns (output, exec_time_ns)."""
    res = _run_on_device(activity, trace=True)
    out = _finish_on_host([r["msum"] for r in res.results])
    return out, res.exec_time_ns
